# revision 1
# baseline (speedup 1.0000x reference)
"""Causal self-attention (RoPE) Trainium2 kernel, 8-way head-parallel.

Sharding: each of the 8 cores computes 2 of the 16 heads for all 4 batches
(tensor parallel over heads: W_qkv column-split, W_proj row-split). Host
pre-transposes x -> xT [C, B*T], slices per-core weights, and sum-reduces the
8 partial projection outputs (+ b_proj) — the standard row-parallel TP reduce.

Per-core dataflow (fp32 storage, fp32r matmuls):
  qkvT = W_slice.T @ xT            [feat, tok] PSUM, bias added on evac
  RoPE on qT,kT                    (rotate-half via permutation matmul on PE)
  v: PE-transpose vT -> v tiles    [tok, d] (+ ones column for softmax denom)
  per (b, h, i-chunk):  S^T tiles = kT_jtile.T @ qT_ichunk   (j on partitions)
      P^T = exp(S^T/8) (ACT), causal mask on diagonal tiles (multiplicative)
      [yT_h | denom] += v_aug.T @ P^T   accumulated over j-tiles in PSUM
  yT_h *= approx_recip(denom bcast via K=1 matmul), out = sum_h yT_h.T @ Wp_h
"""

import numpy as np

import concourse.bass as bass
import concourse.mybir as mybir
import concourse.tile as tile

F32 = mybir.dt.float32
F32R = mybir.dt.float32r
BF16 = mybir.dt.bfloat16
AF = mybir.ActivationFunctionType
OP = mybir.AluOpType

# ---------------------------------------------------------------- tile patch
# This walrus build rejects >1 embedded sync-wait on sync-engine CTRL
# instructions; Tile's tail drain embeds one wait per outstanding semaphore.
# Split them across NOPs (1 wait each) before the drain.


def _patched_drain_and_barrier(self, tick_clock, wait_clock):
    from concourse.tile import ScopedClock

    nc = self.nc
    probe = nc.sync.nop(nofuse=True)
    wait_clock.add_sem_waits(probe.ins, ScopedClock({None: tick_clock.global_clock}))
    si = probe.ins.sync_info
    waits = list(si.on_wait) if si is not None and si.on_wait else []
    if len(waits) > 1:
        si.on_wait = waits[:1]
        for w in waits[1:]:
            nop = nc.sync.nop(nofuse=True)
            nsi = nop.ins.sync_info
            if nsi is None:
                nop.ins.sync_info = mybir.SyncInfo(on_wait=[w], on_update=[])
            else:
                nsi.on_wait = [w]
    nc.sync.drain()
    nc.all_engine_barrier()
    assert self.sems is not None
    popped = nc._tile_sem_poison_stack.pop()
    assert popped is self._sem_poison
    # chunk the sem clears: the range-encoded gpsimd drain (dma_reset) in this
    # walrus build rejects wide semaphore ranges ("ISA wrong length")
    sems = sorted(
        s.num if hasattr(s, "num") else s for s in self.sems.allocated().values()
    )
    for i in range(0, len(sems), 16):
        nc.clear_and_free_semaphores(sems[i : i + 16])
    nc.all_engine_barrier()


tile.TileContext._drain_and_barrier = _patched_drain_and_barrier


def _split_waits(nc):
    """Hoist all but one sync-wait per instruction onto same-engine NOPs
    (this walrus codegen supports a single embedded wait per instruction)."""
    n = 0
    for f in nc.m.functions:
        for blk in f.blocks:
            out = []
            changed = False
            for ins in blk.instructions:
                si = ins.sync_info
                if si is not None and si.on_wait and len(si.on_wait) > 1:
                    waits = list(si.on_wait)
                    for w in waits[:-1]:
                        n += 1
                        out.append(
                            mybir.InstNoOp(
                                name=f"wsplit{n}",
                                engine=ins.engine,
                                sync_info=mybir.SyncInfo(on_wait=[w], on_update=[]),
                                bass_nofuse=True,
                            )
                        )
                    si.on_wait = waits[-1:]
                    changed = True
                out.append(ins)
            if changed:
                blk.instructions = out
    return n

# ------------------------------------------------------------------- builder

HD = 64  # head dim (fixed)
ROPE_BASE = 10000.0


def build_nc(B, T, C, rope_k_engine="gpsimd", mask_engine="vector", debug=False, split=True):
    """One core's program: 2 heads x B batches. T % 512 == 0, C % 128 == 0."""
    assert T % 512 == 0 and C % 128 == 0
    TOK = B * T
    KC = C // 128   # contraction chunks for QKV
    NCH = T // 512  # i-chunks per batch
    NJT = T // 128  # j-tiles per batch
    FW = 128        # qkv feature width per tensor (2 heads * 64)

    nc = bass.Bass()
    xT = nc.dram_tensor("xT", [C, TOK], BF16, kind="ExternalInput")
    wq = nc.dram_tensor("wq", [C, FW], BF16, kind="ExternalInput")
    wk = nc.dram_tensor("wk", [C, FW], BF16, kind="ExternalInput")
    wv = nc.dram_tensor("wv", [C, FW], BF16, kind="ExternalInput")
    bq = nc.dram_tensor("bq", [FW, 1], F32, kind="ExternalInput")
    bk = nc.dram_tensor("bk", [FW, 1], F32, kind="ExternalInput")
    bv = nc.dram_tensor("bv", [FW, 1], F32, kind="ExternalInput")
    wp = nc.dram_tensor("wp", [FW, C], BF16, kind="ExternalInput")
    cosT = nc.dram_tensor("cosT", [FW, T], F32, kind="ExternalInput")
    sinT = nc.dram_tensor("sinT", [FW, T], F32, kind="ExternalInput")
    perm = nc.dram_tensor("perm", [128, 128], BF16, kind="ExternalInput")
    mask4 = nc.dram_tensor("mask4", [128, 4 * 512], BF16, kind="ExternalInput")
    outp = nc.dram_tensor("outp", [TOK, C], BF16, kind="ExternalOutput")
    if debug:
        dbg_q = nc.dram_tensor("dbg_q", [FW, TOK], F32, kind="ExternalOutput")
        dbg_k = nc.dram_tensor("dbg_k", [FW, TOK], F32, kind="ExternalOutput")
        dbg_v = nc.dram_tensor(
            "dbg_v", [B, 128, (T // 128) * (HD + 1)], F32, kind="ExternalOutput"
        )
        dbg_den = nc.dram_tensor("dbg_den", [B, 2 * T], F32, kind="ExternalOutput")
        dbg_y = nc.dram_tensor("dbg_y", [FW, TOK], F32, kind="ExternalOutput")
        dbg_pt = nc.dram_tensor("dbg_pt", [128, 1024], F32, kind="ExternalOutput")

    xT_r = xT[:, :].rearrange("(a p) t -> p a t", p=128)  # [128, KC, TOK]

    rope_eng = {"vector": nc.vector, "gpsimd": nc.gpsimd}[rope_k_engine]
    mask_eng = {"vector": nc.vector, "gpsimd": nc.gpsimd}[mask_engine]

    with tile.TileContext(nc) as tc:
        with (
            tc.tile_pool(name="const", bufs=1) as cpool,
            tc.tile_pool(name="xt", bufs=3) as xpool,
            tc.tile_pool(name="qk", bufs=2) as qkpool,
            tc.tile_pool(name="vv", bufs=2) as vpool,
            tc.tile_pool(name="yy", bufs=2) as ypool,
            tc.tile_pool(name="small", bufs=2) as spool,
            tc.tile_pool(name="den", bufs=3) as dpool,
            tc.tile_pool(name="pt", bufs=4) as ptpool,
            tc.tile_pool(name="outs", bufs=4) as opool,
            tc.tile_pool(name="dram", bufs=2, space="DRAM") as drampool,
            tc.tile_pool(name="ps_mm", bufs=2, space="PSUM") as ps_mm,
            tc.tile_pool(name="ps_s", bufs=2, space="PSUM") as ps_s,
            tc.tile_pool(name="ps_y", bufs=1, space="PSUM") as ps_y,
        ):
            # ---- constants ----
            w_sb = {}
            for name, dram in (("wq", wq), ("wk", wk), ("wv", wv)):
                t = cpool.tile([128, KC, FW], BF16, tag=name)
                nc.sync.dma_start(
                    t[:, :, :], dram[:, :].rearrange("(a p) f -> p a f", p=128)
                )
                w_sb[name] = t
            b_sb = {}
            for name, dram in (("bq", bq), ("bk", bk), ("bv", bv)):
                t = cpool.tile([FW, 1], F32, tag=name)
                nc.sync.dma_start(t[:, :], dram[:, :])
                b_sb[name] = t
            wp2_sb = cpool.tile([FW, C], BF16, tag="wp2")
            nc.sync.dma_start(wp2_sb[:, :], wp[:, :])
            cos_sb = cpool.tile([FW, T], F32, tag="cos")
            nc.sync.dma_start(cos_sb[:, :], cosT[:, :])
            sin_sb = cpool.tile([FW, T], F32, tag="sin")
            nc.sync.dma_start(sin_sb[:, :], sinT[:, :])
            perm_sb = cpool.tile([128, 128], BF16, tag="perm")
            nc.sync.dma_start(perm_sb[:, :], perm[:, :])
            mask_sb = cpool.tile([128, 4 * 512], BF16, tag="mask")
            nc.sync.dma_start(mask_sb[:, :], mask4[:, :])
            ident = cpool.tile([128, 128], BF16, tag="ident")
            from concourse.masks import make_identity

            make_identity(nc, ident[:, :])
            ones_raw = cpool.tile([128, 128], F32, tag="ones_raw")
            nc.vector.memset(ones_raw[:, :], 1.0)
            ones128 = cpool.tile([128, 128], F32, tag="ones128")
            nc.vector.tensor_copy(ones128[:, :].bitcast(F32R), ones_raw[:, :])

            state = {}

            def alloc_qkv(b):
                st = state.setdefault(b, {})
                st["qT"] = qkpool.tile([FW, T], BF16, tag="qT", name=f"qT{b}")
                st["kT"] = qkpool.tile([FW, T], BF16, tag="kT", name=f"kT{b}")
                st["v0"] = vpool.tile([128, NJT, HD + 1], BF16, tag="v0", name=f"v0{b}")
                st["v1"] = vpool.tile([128, NJT, HD + 1], BF16, tag="v1", name=f"v1{b}")
                nc.vector.tensor_copy(
                    st["v0"][:, :, HD], ones_raw[:, 0:1].broadcast_to([128, NJT])
                )
                nc.vector.tensor_copy(
                    st["v1"][:, :, HD], ones_raw[:, 0:1].broadcast_to([128, NJT])
                )

            def emit_qkv_chunk(b, cn):
                st = state[b]
                tok0 = b * T
                qT, kT, v0, v1 = st["qT"], st["kT"], st["v0"], st["v1"]
                ts0 = cn * 512
                xt = xpool.tile([128, KC, 512], BF16, tag="xt", name=f"xt{b}_{cn}")
                nc.sync.dma_start(
                    xt[:, :, :], xT_r[:, :, tok0 + ts0 : tok0 + ts0 + 512]
                )
                for name in ("wq", "wk", "wv"):
                    ps = ps_mm.tile([128, 512], F32, tag="mm", name=f"qkvps{b}_{cn}_{name}")
                    for kc in range(KC):
                        nc.tensor.matmul(
                            ps[:, :],
                            lhsT=w_sb[name][:, kc, :],
                            rhs=xt[:, kc, :],
                            start=(kc == 0),
                            stop=(kc == KC - 1),
                        )
                    if name == "wv":
                        vch = spool.tile([128, 512], BF16, tag="vch", name=f"vch{b}_{cn}")
                        nc.vector.tensor_scalar_add(vch[:, :], ps[:, :], b_sb["bv"][:, :])
                        for qd in range(4):
                            pst = ps_mm.tile([128, 128], BF16, tag="mm", name=f"pst{b}_{cn}_{qd}")
                            nc.tensor.transpose(
                                pst[:, :], vch[:, qd * 128 : qd * 128 + 128], ident[:, :]
                            )
                            jt = cn * 4 + qd
                            nc.vector.tensor_copy(v0[:, jt, 0:HD], pst[:, 0:HD])
                            nc.vector.tensor_copy(v1[:, jt, 0:HD], pst[:, HD:128])
                    else:
                        dest = qT if name == "wq" else kT
                        dch = dest[:, ts0 : ts0 + 512]
                        bias = b_sb["bq" if name == "wq" else "bk"]
                        nc.vector.tensor_scalar_add(dch, ps[:, :], bias[:, :])
                        swp = ps_mm.tile([128, 512], F32, tag="mm", name=f"swp{b}_{cn}_{name}")
                        nc.tensor.matmul(
                            swp[:, :], lhsT=perm_sb[:, :], rhs=dch, start=True, stop=True
                        )
                        cc = cos_sb[:, ts0 : ts0 + 512]
                        ss = sin_sb[:, ts0 : ts0 + 512]
                        t1 = spool.tile([128, 512], F32, tag="t1", name=f"t1{b}_{cn}_{name}")
                        t2 = spool.tile([128, 512], F32, tag="t2", name=f"t2{b}_{cn}_{name}")
                        eng = nc.vector if name == "wq" else rope_eng
                        eng.tensor_tensor(t1[:, :], dch, cc, op=OP.mult)
                        nc.vector.tensor_tensor(t2[:, :], swp[:, :], ss, op=OP.mult)
                        eng.tensor_tensor(dch, t1[:, :], t2[:, :], op=OP.add)

            def alloc_attn(b):
                st = state[b]
                st["yT0"] = ypool.tile([HD, T], BF16, tag="yT0", name=f"yT0{b}")
                st["yT1"] = ypool.tile([HD, T], BF16, tag="yT1", name=f"yT1{b}")
                st["yTfull"] = ypool.tile([FW, T], BF16, tag="yTfull", name=f"yTfull{b}")
                st["den_allA"] = spool.tile([68, 512], F32, tag="den_allA", name=f"den_allA{b}")
                st["den_allB"] = spool.tile([68, 512], F32, tag="den_allB", name=f"den_allB{b}")
                st["rec_allA"] = spool.tile([68, 512], F32, tag="rec_allA", name=f"rec_allA{b}")
                st["rec_allB"] = spool.tile([68, 512], F32, tag="rec_allB", name=f"rec_allB{b}")
                if debug:
                    nc.gpsimd.dma_start(
                        dbg_q[:, b * T : b * T + T], st["qT"][:, :]
                    )
                    nc.gpsimd.dma_start(
                        dbg_k[:, b * T : b * T + T], st["kT"][:, :]
                    )
                    nc.gpsimd.dma_start(
                        dbg_v[b, :, :], st["v0"][:, :, :].rearrange("p a d -> p (a d)")
                    )

            def emit_attn_ic(b, ic):
                st = state[b]
                qT, kT = st["qT"], st["kT"]
                vh = {0: st["v0"], 1: st["v1"]}
                yTh = {0: st["yT0"], 1: st["yT1"]}
                i0 = ic * 512
                njt = (ic + 1) * 4
                yps = {}
                for h in range(2):
                    yps[h] = ps_y.tile([HD + 1, 512], F32, tag=f"y{h}", name=f"yps{b}_{ic}_{h}")
                for jg in range((njt + 1) // 2):
                    jts = [jt for jt in (2 * jg, 2 * jg + 1) if jt < njt]
                    sps = {}
                    pt = {}
                    for h in range(2):
                        hr0 = h * HD
                        sps[h] = ps_s.tile([128, 1024], F32, tag="s", name=f"sps{b}_{ic}_{jg}_{h}")
                        for li, jt in enumerate(jts):
                            nc.tensor.matmul(
                                sps[h][:, li * 512 : li * 512 + 512],
                                lhsT=kT[hr0 : hr0 + HD, jt * 128 : jt * 128 + 128],
                                rhs=qT[hr0 : hr0 + HD, i0 : i0 + 512],
                                start=True,
                                stop=True,
                            )
                    for h in range(2):
                        pt[h] = ptpool.tile([128, 1024], BF16, tag="pt", name=f"pt{b}_{ic}_{jg}_{h}")
                        nc.scalar.activation(
                            pt[h][:, 0 : len(jts) * 512],
                            sps[h][:, 0 : len(jts) * 512],
                            AF.Exp,
                            scale=float(1.0 / np.sqrt(HD)),
                        )
                    for h in range(2):
                        for li, jt in enumerate(jts):
                            ptt = pt[h][:, li * 512 : li * 512 + 512]
                            dv = jt - (njt - 4)
                            if dv >= 0:
                                mask_eng.tensor_tensor(
                                    ptt,
                                    ptt,
                                    mask_sb[:, dv * 512 : dv * 512 + 512],
                                    op=OP.mult,
                                )
                            if debug and b == 0 and ic == 0 and h == 0 and jg == 0:
                                nc.gpsimd.dma_start(
                                    dbg_pt[:, li * 512 : li * 512 + 512], ptt
                                )
                            nc.tensor.matmul(
                                yps[h][:, :],
                                lhsT=vh[h][:, jt, :],
                                rhs=ptt,
                                start=(jt == 0),
                                stop=(jt == njt - 1),
                                skip_group_check=True,
                            )
                for h in range(2):
                    nc.vector.tensor_copy(yTh[h][:, i0 : i0 + 512], yps[h][0:HD, :])
                    den_t = dpool.tile([HD + 1, 512], F32, tag="den", name=f"den{b}_{ic}_{h}")
                    nc.vector.tensor_copy(den_t[HD : HD + 1, :], yps[h][HD : HD + 1, :])
                    if debug:
                        nc.gpsimd.dma_start(
                            dbg_den[b : b + 1, h * T + i0 : h * T + i0 + 512],
                            den_t[HD : HD + 1, :],
                        )
                    half = "A" if ic < NCH // 2 else "B"
                    r = (ic % (NCH // 2)) * 2 + h
                    nc.sync.dma_start(
                        st["den_all" + half][HD + r : HD + r + 1, :],
                        den_t[HD : HD + 1, :],
                    )

            def emit_recip_half(b, half):
                st = state[b]
                if "rec_dram" not in st:
                    st["rec_dram"] = drampool.tile(
                        [2 * NCH, 512], F32, tag="rec_dram", name=f"rec_dram{b}"
                    )
                rd = st["rec_dram"]
                n = NCH  # rows per half (2 heads * NCH/2 ics)
                nc.vector.reciprocal(
                    st["rec_all" + half][HD : HD + n, :],
                    st["den_all" + half][HD : HD + n, :],
                )
                off = 0 if half == "A" else n
                nc.sync.dma_start(
                    rd[off : off + n, :], st["rec_all" + half][HD : HD + n, :]
                )
                if debug and half == "B":
                    nc.gpsimd.dma_start(
                        dbg_y[0:HD, b * T : b * T + T], st["yT0"][:, :]
                    )
                    nc.gpsimd.dma_start(
                        dbg_y[HD:FW, b * T : b * T + T], st["yT1"][:, :]
                    )

            def emit_proj_ic(b, ic):
                st = state[b]
                tok0 = b * T
                yTh = {0: st["yT0"], 1: st["yT1"]}
                yTfull = st["yTfull"]
                rec_dram = st["rec_dram"]
                i0 = ic * 512
                fw = min(512, C)
                rb = spool.tile([128, 512], F32, tag="rb", name=f"rb{b}_{ic}")
                ro = (0 if ic < NCH // 2 else NCH) + (ic % (NCH // 2)) * 2
                nc.sync.dma_start(
                    rb[0:HD, :],
                    rec_dram[ro : ro + 1, :].broadcast_to([HD, 512]),
                )
                nc.sync.dma_start(
                    rb[HD:128, :],
                    rec_dram[ro + 1 : ro + 2, :].broadcast_to([HD, 512]),
                )
                yfp = ps_mm.tile([128, 512], F32, tag="mm", name=f"yfp{b}_{ic}")
                nc.tensor.matmul(
                    yfp[0:HD, :],
                    lhsT=ident[0:HD, 0:HD],
                    rhs=yTh[0][:, i0 : i0 + 512],
                    start=True,
                    stop=True,
                )
                nc.tensor.matmul(
                    yfp[HD:128, :],
                    lhsT=ident[0:HD, 0:HD],
                    rhs=yTh[1][:, i0 : i0 + 512],
                    start=True,
                    stop=True,
                    tile_position=(0, HD),
                )
                nc.vector.tensor_tensor(
                    yTfull[:, i0 : i0 + 512], yfp[:, :], rb[:, :], op=OP.mult
                )
                for tt in range(4):
                    tr0 = i0 + tt * 128
                    for fc in range(C // fw):
                        pp = ps_mm.tile([128, fw], F32, tag="mm", name=f"pp{b}_{ic}_{tt}_{fc}")
                        nc.tensor.matmul(
                            pp[:, :],
                            lhsT=yTfull[:, tr0 : tr0 + 128],
                            rhs=wp2_sb[:, fc * fw : fc * fw + fw],
                            start=True,
                            stop=True,
                        )
                        ot = opool.tile([128, fw], BF16, tag="ot", name=f"ot{b}_{ic}_{tt}_{fc}")
                        if tt % 2 == 0:
                            nc.vector.tensor_copy(ot[:, :], pp[:, :])
                        else:
                            nc.scalar.activation(ot[:, :], pp[:, :], AF.Copy)
                        nc.sync.dma_start(
                            outp[
                                tok0 + tr0 : tok0 + tr0 + 128, fc * fw : fc * fw + fw
                            ],
                            ot[:, :],
                        )

            # ---- software-pipelined emission: qkv(b+1) and proj(b-1)
            # interleave with attention(b) to keep the PE stream dense ----
            alloc_qkv(0)
            for cn in range(NCH):
                emit_qkv_chunk(0, cn)
            for b in range(B):
                alloc_attn(b)
                if b + 1 < B:
                    alloc_qkv(b + 1)
                for ic in range(NCH):
                    emit_attn_ic(b, ic)
                    if b + 1 < B:
                        emit_qkv_chunk(b + 1, ic)
                    if b >= 1:
                        emit_proj_ic(b - 1, ic)
                emit_recip_half(b, "A")
                emit_recip_half(b, "B")
            for ic in range(NCH):
                emit_proj_ic(B - 1, ic)
    if split:
        _split_waits(nc)
    return nc


# ---------------------------------------------------------------- host side


def make_tables(T):
    inv_freq = 1.0 / (ROPE_BASE ** (np.arange(0, HD, 2, dtype=np.float32) / HD))
    pos = np.arange(T, dtype=np.float32)
    freqs = pos[:, None] * inv_freq[None, :]  # [T, 32]
    cos = np.cos(freqs).astype(np.float32)  # [T, 32] (same for both halves)
    sin = np.sin(freqs).astype(np.float32)
    cosT64 = np.concatenate([cos.T, cos.T], axis=0)  # [64, T]
    sinT64 = np.concatenate([-sin.T, sin.T], axis=0)  # sign-baked rotate_half
    cosT = np.concatenate([cosT64, cosT64], axis=0).copy()  # [128, T] two heads
    sinT = np.concatenate([sinT64, sinT64], axis=0).copy()
    return cosT, sinT


def make_perm():
    # perm[k, m] = 1 iff m == (k+32) % 64 within each 64-row head block
    p = np.zeros((128, 128), dtype=np.float32)
    for hb in range(2):
        for k in range(HD):
            p[hb * HD + k, hb * HD + (k + 32) % HD] = 1.0
    return p


def make_mask4():
    # mask4[p, v*512 + f] = 1.0 if v*128 + p <= f else 0.0
    m = np.zeros((128, 4 * 512), dtype=np.float32)
    p = np.arange(128)[:, None]
    f = np.arange(512)[None, :]
    for v in range(4):
        m[:, v * 512 : (v + 1) * 512] = (v * 128 + p <= f).astype(np.float32)
    return m


def make_in_maps(x, W_qkv, b_qkv, W_proj, n_cores):
    B, T, C = x.shape
    import ml_dtypes

    xT = np.ascontiguousarray(x.reshape(B * T, C).T.astype(ml_dtypes.bfloat16))
    cosT, sinT = make_tables(T)
    mask4 = make_mask4()
    perm = make_perm()
    in_maps = []
    for c in range(n_cores):
        h0 = 2 * c * HD  # first head's column offset (2 heads per core)
        sl = slice(h0, h0 + 128)
        in_maps.append(
            {
                "xT": xT,
                "wq": np.ascontiguousarray(W_qkv[:, sl].astype(ml_dtypes.bfloat16)),
                "wk": np.ascontiguousarray(
                    W_qkv[:, C:][:, sl].astype(ml_dtypes.bfloat16)
                ),
                "wv": np.ascontiguousarray(
                    W_qkv[:, 2 * C :][:, sl].astype(ml_dtypes.bfloat16)
                ),
                "bq": np.ascontiguousarray(b_qkv[sl].reshape(128, 1)),
                "bk": np.ascontiguousarray(b_qkv[C:][sl].reshape(128, 1)),
                "bv": np.ascontiguousarray(b_qkv[2 * C :][sl].reshape(128, 1)),
                "wp": np.ascontiguousarray(W_proj[sl, :].astype(ml_dtypes.bfloat16)),
                "cosT": cosT,
                "sinT": sinT,
                "perm": perm.astype(ml_dtypes.bfloat16),
                "mask4": mask4.astype(ml_dtypes.bfloat16),
            }
        )
    return in_maps


_NC_CACHE = {}


def _get_nc(B, T, C):
    key = (B, T, C)
    if key not in _NC_CACHE:
        _NC_CACHE[key] = build_nc(B, T, C)
    return _NC_CACHE[key]


def kernel(x, W_qkv, b_qkv, W_proj, b_proj):
    from concourse.bass_utils import run_bass_kernel_spmd

    x = np.asarray(x, dtype=np.float32)
    W_qkv = np.asarray(W_qkv, dtype=np.float32)
    b_qkv = np.asarray(b_qkv, dtype=np.float32)
    W_proj = np.asarray(W_proj, dtype=np.float32)
    b_proj = np.asarray(b_proj, dtype=np.float32)
    B, T, C = x.shape
    n_cores = 8
    nc = _get_nc(B, T, C)
    in_maps = make_in_maps(x, W_qkv, b_qkv, W_proj, n_cores)
    res = run_bass_kernel_spmd(nc, in_maps, core_ids=list(range(n_cores)))
    out = np.zeros((B * T, C), dtype=np.float32)
    for r in res.results:
        out += r["outp"].astype(np.float32)
    out += b_proj[None, :]
    return out.reshape(B, T, C)



# revision 9
# speedup vs baseline: 1.2365x; 1.2365x over previous
"""Causal self-attention (RoPE) Trainium2 kernel, 8-way head-parallel.

Sharding: each of the 8 cores computes 2 of the 16 heads for all 4 batches
(tensor parallel over heads: W_qkv column-split, W_proj row-split). Host
pre-transposes x -> xT [C, B*T], slices per-core weights, and sum-reduces the
8 partial projection outputs (+ b_proj) — the standard row-parallel TP reduce.

Per-core dataflow (bf16 storage/matmuls, fp32 PSUM):
  qkvT = W_slice.T @ xT            [feat, tok] PSUM, bias added on ACT evac
  RoPE on qT,kT                    (rotate-half via permutation matmul on PE)
  v: PE-transpose vT -> vc tiles   [tok, 2*(HD+1)] (+ ones col per head)
  per (b, h, i-chunk, jg of 2 j-tiles):
      S^T = kT_jt.T @ qT_ichunk    (j on partitions)
      P^T = exp(S^T/8) (ACT), causal mask on diagonal tiles (mult, DVE)
      [yT_h | denom] += vc_h.T @ P^T   accumulated over j-tiles in PSUM
  yT_h *= recip(denom) (approx recip, bf16 bcast), out = sum_h yT_h.T @ Wp_h

The emission schedule software-pipelines at j-group granularity: between a
j-group's S matmuls and the PREVIOUS group's AV matmuls we pop one "filler"
unit (a QKV weight-stream for batch b+1 or a projection half for an earlier
chunk) so the PE stream stays dense while ACT computes exp.
"""

from collections import deque

import numpy as np

import concourse.bass as bass
import concourse.mybir as mybir
import concourse.tile as tile

F32 = mybir.dt.float32
BF16 = mybir.dt.bfloat16
AF = mybir.ActivationFunctionType
OP = mybir.AluOpType

# ---------------------------------------------------------------- tile patch
# This walrus build rejects >1 embedded sync-wait on sync-engine CTRL
# instructions; Tile's tail drain embeds one wait per outstanding semaphore.
# Split them across NOPs (1 wait each) before the drain.


def _patched_drain_and_barrier(self, tick_clock, wait_clock):
    from concourse.tile import ScopedClock

    nc = self.nc
    probe = nc.sync.nop(nofuse=True)
    wait_clock.add_sem_waits(probe.ins, ScopedClock({None: tick_clock.global_clock}))
    si = probe.ins.sync_info
    waits = list(si.on_wait) if si is not None and si.on_wait else []
    if len(waits) > 1:
        si.on_wait = waits[:1]
        for w in waits[1:]:
            nop = nc.sync.nop(nofuse=True)
            nsi = nop.ins.sync_info
            if nsi is None:
                nop.ins.sync_info = mybir.SyncInfo(on_wait=[w], on_update=[])
            else:
                nsi.on_wait = [w]
    nc.sync.drain()
    nc.all_engine_barrier()
    assert self.sems is not None
    popped = nc._tile_sem_poison_stack.pop()
    assert popped is self._sem_poison
    # chunk the sem clears: the range-encoded gpsimd drain (dma_reset) in this
    # walrus build rejects wide semaphore ranges ("ISA wrong length")
    sems = sorted(
        s.num if hasattr(s, "num") else s for s in self.sems.allocated().values()
    )
    for i in range(0, len(sems), 16):
        nc.clear_and_free_semaphores(sems[i : i + 16])
    nc.all_engine_barrier()


tile.TileContext._drain_and_barrier = _patched_drain_and_barrier


def _split_waits(nc):
    """Hoist all but one sync-wait per instruction onto same-engine NOPs
    (this walrus codegen supports a single embedded wait per instruction)."""
    n = 0
    for f in nc.m.functions:
        for blk in f.blocks:
            out = []
            changed = False
            for ins in blk.instructions:
                si = ins.sync_info
                if si is not None and si.on_wait and len(si.on_wait) > 1:
                    waits = list(si.on_wait)
                    for w in waits[:-1]:
                        n += 1
                        out.append(
                            mybir.InstNoOp(
                                name=f"wsplit{n}",
                                engine=ins.engine,
                                sync_info=mybir.SyncInfo(on_wait=[w], on_update=[]),
                                bass_nofuse=True,
                            )
                        )
                    si.on_wait = waits[-1:]
                    changed = True
                out.append(ins)
            if changed:
                blk.instructions = out
    return n


# ------------------------------------------------------------------- builder

HD = 64  # head dim (fixed)
ROPE_BASE = 10000.0


def build_nc(B, T, C, split=True):
    """One core's program: 2 heads x B batches. T % 512 == 0, C % 128 == 0."""
    assert T % 512 == 0 and C % 128 == 0
    TOK = B * T
    KC = C // 128   # contraction chunks for QKV
    NCH = T // 512  # i-chunks per batch
    NJT = T // 128  # j-tiles per batch
    FW = 128        # qkv feature width per tensor (2 heads * 64)
    HALF = NCH // 2

    nc = bass.Bass()
    xT = nc.dram_tensor("xT", [C, TOK], BF16, kind="ExternalInput")
    wq = nc.dram_tensor("wq", [C, FW], BF16, kind="ExternalInput")
    wk = nc.dram_tensor("wk", [C, FW], BF16, kind="ExternalInput")
    wv = nc.dram_tensor("wv", [C, FW], BF16, kind="ExternalInput")
    bq = nc.dram_tensor("bq", [FW, 1], F32, kind="ExternalInput")
    bk = nc.dram_tensor("bk", [FW, 1], F32, kind="ExternalInput")
    bv = nc.dram_tensor("bv", [FW, 1], F32, kind="ExternalInput")
    wp = nc.dram_tensor("wp", [FW, C], BF16, kind="ExternalInput")
    cosT = nc.dram_tensor("cosT", [FW, T], BF16, kind="ExternalInput")
    sinT = nc.dram_tensor("sinT", [FW, T], BF16, kind="ExternalInput")
    perm = nc.dram_tensor("perm", [128, 128], BF16, kind="ExternalInput")
    mask4 = nc.dram_tensor("mask4", [128, 4 * 512], BF16, kind="ExternalInput")
    outp = nc.dram_tensor("outp", [TOK, C], BF16, kind="ExternalOutput")

    xT_r = xT[:, :].rearrange("(a p) t -> p a t", p=128)  # [128, KC, TOK]

    with tile.TileContext(nc) as tc:
        with (
            tc.tile_pool(name="const", bufs=1) as cpool,
            tc.tile_pool(name="xt", bufs=3) as xpool,
            tc.tile_pool(name="qk", bufs=2) as qkpool,
            tc.tile_pool(name="vv", bufs=2) as vpool,
            tc.tile_pool(name="yy", bufs=2) as ypool,
            tc.tile_pool(name="small", bufs=2) as spool,
            tc.tile_pool(name="den", bufs=2) as dpool,
            tc.tile_pool(name="pt", bufs=4) as ptpool,
            tc.tile_pool(name="outs", bufs=4) as opool,
            tc.tile_pool(name="dram", bufs=2, space="DRAM") as drampool,
            tc.tile_pool(name="ps_mm", bufs=2, space="PSUM") as ps_mm,
            tc.tile_pool(name="ps_s", bufs=2, space="PSUM") as ps_s,
            tc.tile_pool(name="ps_y", bufs=1, space="PSUM") as ps_y,
        ):
            # ---- constants (priority order: first QKV chunk's deps first) ----
            w_sb = {}
            b_sb = {}
            for name, dram, bias_d in (("wq", wq, bq), ("wk", wk, bk), ("wv", wv, bv)):
                t = cpool.tile([128, KC, FW], BF16, tag=name)
                for g in range(0, KC, KC // 2):
                    nc.sync.dma_start(
                        t[:, g : g + KC // 2, :],
                        dram[:, :].rearrange("(a p) f -> p a f", p=128)[
                            :, g : g + KC // 2, :
                        ],
                    )
                w_sb[name] = t
                bname = "b" + name[1]
                bt = cpool.tile([FW, 1], F32, tag=bname)
                nc.sync.dma_start(bt[:, :], bias_d[:, :])
                b_sb[bname] = bt
            perm_sb = cpool.tile([128, 128], BF16, tag="perm")
            nc.sync.dma_start(perm_sb[:, :], perm[:, :])
            cos_sb = cpool.tile([FW, T], BF16, tag="cos")
            sin_sb = cpool.tile([FW, T], BF16, tag="sin")
            for g in range(0, T, T // 2):
                nc.sync.dma_start(cos_sb[:, g : g + T // 2], cosT[:, g : g + T // 2])
                nc.sync.dma_start(sin_sb[:, g : g + T // 2], sinT[:, g : g + T // 2])
            mask_sb = cpool.tile([128, 4 * 512], BF16, tag="mask")
            nc.sync.dma_start(mask_sb[:, :], mask4[:, :])
            wp2_sb = cpool.tile([FW, C], BF16, tag="wp2")
            nc.sync.dma_start(wp2_sb[:, :], wp[:, :])
            ident = cpool.tile([128, 128], BF16, tag="ident")
            from concourse.masks import make_identity

            make_identity(nc, ident[:, :])
            ones_raw = cpool.tile([128, 128], F32, tag="ones_raw")
            nc.vector.memset(ones_raw[:, :], 1.0)

            state = {}
            deferred = []  # PE/DVE ops deferred one unit so PE never waits ACT

            def run_deferred():
                while deferred:
                    deferred.pop(0)()

            def alloc_qkv(b):
                st = state.setdefault(b, {})
                st["qT"] = qkpool.tile([FW, T], BF16, tag="qT", name=f"qT{b}")
                st["kT"] = qkpool.tile([FW, T], BF16, tag="kT", name=f"kT{b}")
                # v combined: per j-tile, [v_h0 (64) | ones | v_h1 (64) | ones]
                st["vc"] = vpool.tile([128, NJT, 2 * (HD + 1)], BF16, tag="vc",
                                      name=f"vc{b}")
                nc.vector.tensor_copy(
                    st["vc"][:, :, HD], ones_raw[:, 0:1].broadcast_to([128, NJT])
                )
                nc.vector.tensor_copy(
                    st["vc"][:, :, 2 * HD + 1],
                    ones_raw[:, 0:1].broadcast_to([128, NJT]),
                )

            def unit_qkv(b, cn, name):
                def fn():
                    run_deferred()
                    if "qT" not in state.setdefault(b, {}):
                        alloc_qkv(b)
                    st = state[b]
                    tok0 = b * T
                    ts0 = cn * 512
                    if name == "wq":
                        xt = xpool.tile([128, KC, 512], BF16, tag="xt",
                                        name=f"xt{b}_{cn}")
                        st[("xt", cn)] = xt
                        for g in range(0, KC, 2):
                            nc.sync.dma_start(
                                xt[:, g : g + 2, :],
                                xT_r[:, g : g + 2, tok0 + ts0 : tok0 + ts0 + 512],
                            )
                    xt = st[("xt", cn)]
                    ps = ps_mm.tile([128, 512], F32, tag="mm",
                                    name=f"qkvps{b}_{cn}_{name}")
                    for kc in range(KC):
                        nc.tensor.matmul(
                            ps[:, :],
                            lhsT=w_sb[name][:, kc, :],
                            rhs=xt[:, kc, :],
                            start=(kc == 0),
                            stop=(kc == KC - 1),
                        )
                    if name == "wv":
                        st.pop(("xt", cn))
                        vch = spool.tile([128, 512], BF16, tag="vch",
                                         name=f"vch{b}_{cn}")
                        nc.scalar.activation(
                            vch[:, :], ps[:, :], AF.Identity, bias=b_sb["bv"][:, :]
                        )

                        def dtrans():
                            vc = state[b]["vc"]
                            for qd in range(4):
                                pst = ps_mm.tile([128, 128], BF16, tag="mm",
                                                 name=f"pst{b}_{cn}_{qd}")
                                nc.tensor.transpose(
                                    pst[:, :],
                                    vch[:, qd * 128 : qd * 128 + 128],
                                    ident[:, :],
                                )
                                jt = cn * 4 + qd
                                nc.vector.tensor_copy(
                                    vc[:, jt, :].rearrange(
                                        "p (a c) -> p a c", a=2
                                    )[:, :, 0:HD],
                                    pst[:, :].rearrange("p (a c) -> p a c", a=2),
                                )

                        deferred.append(dtrans)
                    else:
                        dest = st["qT"] if name == "wq" else st["kT"]
                        dch = dest[:, ts0 : ts0 + 512]
                        bias = b_sb["bq" if name == "wq" else "bk"]
                        nc.scalar.activation(dch, ps[:, :], AF.Identity, bias=bias[:, :])

                        def drope():
                            swp = ps_mm.tile([128, 512], F32, tag="mm",
                                             name=f"swp{b}_{cn}_{name}")
                            nc.tensor.matmul(
                                swp[:, :], lhsT=perm_sb[:, :], rhs=dch,
                                start=True, stop=True,
                            )
                            cc = cos_sb[:, ts0 : ts0 + 512]
                            ss = sin_sb[:, ts0 : ts0 + 512]
                            t1 = spool.tile([128, 512], F32, tag="t1",
                                            name=f"t1{b}_{cn}_{name}")
                            t2 = spool.tile([128, 512], F32, tag="t2",
                                            name=f"t2{b}_{cn}_{name}")
                            eng = nc.vector if name == "wq" else nc.gpsimd
                            eng.tensor_tensor(t1[:, :], dch, cc, op=OP.mult)
                            nc.vector.tensor_tensor(t2[:, :], swp[:, :], ss,
                                                    op=OP.mult)
                            eng.tensor_tensor(dch, t1[:, :], t2[:, :], op=OP.add)

                        deferred.append(drope)

                return fn

            # ---- filler machinery ----
            pending = deque()   # (b, cn, name, fn) in emission order
            proj_q = deque()    # (b, fn)

            def pop_filler(cur_b):
                if pending and pending[0][0] <= cur_b + 1:
                    pending.popleft()[3]()
                elif proj_q:
                    proj_q.popleft()[1]()

            def force_qkv(b, cn):
                while pending and (
                    pending[0][0] < b
                    or (pending[0][0] == b and pending[0][1] <= cn)
                ):
                    pending.popleft()[3]()

            def drain_proj(max_b):
                while proj_q and proj_q[0][0] <= max_b:
                    proj_q.popleft()[1]()

            # ---- attention ----
            def attn_begin(b, ic):
                st = state[b]
                yps = {
                    h: ps_y.tile([HD + 1, 512], F32, tag=f"y{h}",
                                 name=f"yps{b}_{ic}_{h}")
                    for h in range(2)
                }
                st[("yps", ic)] = yps
                if ic == 0:
                    st["yTfull"] = ypool.tile([FW, T], BF16, tag="yTfull",
                                              name=f"yTfull{b}")
                    # den rows: half A (ics 0..HALF-1) at partitions 64..,
                    # half B at 96.. — 32-aligned starts for the custom DVE
                    # reciprocal.
                    st["den_all"] = dpool.tile([96 + NCH, 512], F32,
                                               tag="den", name=f"den{b}")
                    st["rec_all"] = dpool.tile([96 + NCH, 512], F32,
                                               tag="rec", name=f"rec{b}")
                    st["rec_bf"] = dpool.tile([96 + NCH, 512], BF16,
                                              tag="recbf", name=f"recbf{b}")
                    st["rec_dram"] = drampool.tile([2 * NCH, 512], BF16,
                                                   tag="rec_dram",
                                                   name=f"rec_dram{b}")

            def emit_S(b, ic, jg):
                st = state[b]
                i0 = ic * 512
                sps = {}
                for h in range(2):
                    hr0 = h * HD
                    sp = ps_s.tile([128, 1024], F32, tag="s",
                                   name=f"sps{b}_{ic}_{jg}_{h}")
                    for li in range(2):
                        jt = 2 * jg + li
                        nc.tensor.matmul(
                            sp[:, li * 512 : li * 512 + 512],
                            lhsT=st["kT"][hr0 : hr0 + HD,
                                          jt * 128 : jt * 128 + 128],
                            rhs=st["qT"][hr0 : hr0 + HD, i0 : i0 + 512],
                            start=True,
                            stop=True,
                        )
                    sps[h] = sp
                st[("sps", ic, jg)] = sps

            def emit_exp_mask(b, ic, jg):
                st = state[b]
                njt = 4 * (ic + 1)
                sps = st.pop(("sps", ic, jg))
                pt = {}
                for h in range(2):
                    p = ptpool.tile([128, 1024], BF16, tag="pt",
                                    name=f"pt{b}_{ic}_{jg}_{h}")
                    nc.scalar.activation(
                        p[:, :], sps[h][:, :], AF.Exp,
                        scale=float(1.0 / np.sqrt(HD)),
                    )
                    pt[h] = p
                dv0 = 2 * jg - (njt - 4)
                if dv0 >= 0:  # both j-tiles of this group are diagonal
                    for h in range(2):
                        nc.vector.tensor_tensor(
                            pt[h][:, :], pt[h][:, :],
                            mask_sb[:, dv0 * 512 : dv0 * 512 + 1024],
                            op=OP.mult,
                        )
                st[("pt", ic, jg)] = pt

            def emit_AV(b, ic, jg):
                st = state[b]
                njt = 4 * (ic + 1)
                pt = st.pop(("pt", ic, jg))
                yps = st[("yps", ic)]
                for h in range(2):
                    for li in range(2):
                        jt = 2 * jg + li
                        nc.tensor.matmul(
                            yps[h][:, :],
                            lhsT=st["vc"][:, jt, h * (HD + 1) : (h + 1) * (HD + 1)],
                            rhs=pt[h][:, li * 512 : li * 512 + 512],
                            start=(jt == 0),
                            stop=(jt == njt - 1),
                            skip_group_check=True,
                        )

            def attn_end(b, ic):
                st = state[b]
                yps = st.pop(("yps", ic))
                i0 = ic * 512
                yTfull = st["yTfull"]
                nc.vector.tensor_copy(yTfull[0:HD, i0 : i0 + 512], yps[0][0:HD, :])
                ytmp = spool.tile([HD, 512], BF16, tag="ytmp", name=f"ytmp{b}_{ic}")
                nc.vector.tensor_copy(ytmp[:, :], yps[1][0:HD, :])
                nc.sync.dma_start(yTfull[HD:FW, i0 : i0 + 512], ytmp[:, :])
                dent = spool.tile([HD + 1, 1024], F32, tag="dent",
                                  name=f"dent{b}_{ic}")
                base = (64 if ic < HALF else 96) + (ic % HALF) * 2
                for h in range(2):
                    nc.vector.tensor_copy(
                        dent[HD : HD + 1, h * 512 : h * 512 + 512],
                        yps[h][HD : HD + 1, :],
                    )
                    nc.sync.dma_start(
                        st["den_all"][base + h : base + h + 1, :],
                        dent[HD : HD + 1, h * 512 : h * 512 + 512],
                    )

            def emit_recip_half(b, half):
                st = state[b]
                yTfull = st["yTfull"]
                base = 64 if half == 0 else 96
                off = half * NCH
                n = 2 * HALF
                nc.vector.reciprocal(
                    st["rec_all"][base : base + n, :],
                    st["den_all"][base : base + n, :],
                )
                nc.vector.tensor_copy(
                    st["rec_bf"][base : base + n, :],
                    st["rec_all"][base : base + n, :],
                )
                nc.sync.dma_start(
                    st["rec_dram"][off : off + n, :],
                    st["rec_bf"][base : base + n, :],
                )
                for ic in range(half * HALF, (half + 1) * HALF):
                    i0 = ic * 512
                    ro = off + (ic % HALF) * 2
                    rb = spool.tile([128, 512], BF16, tag="rb", name=f"rb{b}_{ic}")
                    nc.sync.dma_start(
                        rb[0:HD, :],
                        st["rec_dram"][ro : ro + 1, :].broadcast_to([HD, 512]),
                    )
                    nc.sync.dma_start(
                        rb[HD:128, :],
                        st["rec_dram"][ro + 1 : ro + 2, :].broadcast_to([HD, 512]),
                    )
                    nc.vector.tensor_tensor(
                        yTfull[:, i0 : i0 + 512], yTfull[:, i0 : i0 + 512],
                        rb[:, :], op=OP.mult,
                    )

            def unit_proj(b, ic, half):
                def fn():
                    st = state[b]
                    tok0 = b * T
                    i0 = ic * 512
                    yTfull = st["yTfull"]
                    for tt in ((0, 1) if half == 0 else (2, 3)):
                        tr0 = i0 + tt * 128
                        for fc in range(C // 512):
                            pp = ps_mm.tile([128, 512], F32, tag="mm",
                                            name=f"pp{b}_{ic}_{tt}_{fc}")
                            nc.tensor.matmul(
                                pp[:, :],
                                lhsT=yTfull[:, tr0 : tr0 + 128],
                                rhs=wp2_sb[:, fc * 512 : fc * 512 + 512],
                                start=True,
                                stop=True,
                            )
                            ot = opool.tile([128, 512], BF16, tag="ot",
                                            name=f"ot{b}_{ic}_{tt}_{fc}")
                            if tt % 2 == 0:
                                nc.vector.tensor_copy(ot[:, :], pp[:, :])
                            else:
                                nc.scalar.activation(ot[:, :], pp[:, :], AF.Copy)
                            nc.sync.dma_start(
                                outp[tok0 + tr0 : tok0 + tr0 + 128,
                                     fc * 512 : fc * 512 + 512],
                                ot[:, :],
                            )

                return fn

            # ---- schedule ----
            for b in range(B):
                for cn in range(NCH):
                    for name in ("wq", "wk", "wv"):
                        pending.append((b, cn, name, unit_qkv(b, cn, name)))

            for b in range(B):
                for ic in range(NCH):
                    force_qkv(b, ic)
                    run_deferred()
                    if ic == 0:
                        drain_proj(b - 2)
                    attn_begin(b, ic)
                    njg = 2 * (ic + 1)
                    prev = None
                    for jg in range(njg):
                        emit_S(b, ic, jg)
                        emit_exp_mask(b, ic, jg)
                        pop_filler(b)
                        if prev is not None:
                            emit_AV(b, ic, prev)
                        prev = jg
                    pop_filler(b)
                    emit_AV(b, ic, prev)
                    attn_end(b, ic)
                    if ic % HALF == HALF - 1:
                        half = ic // HALF
                        emit_recip_half(b, half)
                        for pic in range(half * HALF, (half + 1) * HALF):
                            proj_q.append((b, unit_proj(b, pic, 0)))
                            proj_q.append((b, unit_proj(b, pic, 1)))
            run_deferred()
            while pending:
                pending.popleft()[3]()
            while proj_q:
                proj_q.popleft()[1]()
    if split:
        _split_waits(nc)
    return nc


# ---------------------------------------------------------------- host side


def make_tables(T):
    inv_freq = 1.0 / (ROPE_BASE ** (np.arange(0, HD, 2, dtype=np.float32) / HD))
    pos = np.arange(T, dtype=np.float32)
    freqs = pos[:, None] * inv_freq[None, :]  # [T, 32]
    cos = np.cos(freqs).astype(np.float32)  # [T, 32] (same for both halves)
    sin = np.sin(freqs).astype(np.float32)
    cosT64 = np.concatenate([cos.T, cos.T], axis=0)  # [64, T]
    sinT64 = np.concatenate([-sin.T, sin.T], axis=0)  # sign-baked rotate_half
    cosT = np.concatenate([cosT64, cosT64], axis=0).copy()  # [128, T] two heads
    sinT = np.concatenate([sinT64, sinT64], axis=0).copy()
    return cosT, sinT


def make_perm():
    # perm[k, m] = 1 iff m == (k+32) % 64 within each 64-row head block
    p = np.zeros((128, 128), dtype=np.float32)
    for hb in range(2):
        for k in range(HD):
            p[hb * HD + k, hb * HD + (k + 32) % HD] = 1.0
    return p


def make_mask4():
    # mask4[p, v*512 + f] = 1.0 if v*128 + p <= f else 0.0
    m = np.zeros((128, 4 * 512), dtype=np.float32)
    p = np.arange(128)[:, None]
    f = np.arange(512)[None, :]
    for v in range(4):
        m[:, v * 512 : (v + 1) * 512] = (v * 128 + p <= f).astype(np.float32)
    return m


def make_in_maps(x, W_qkv, b_qkv, W_proj, n_cores):
    B, T, C = x.shape
    import ml_dtypes

    xT = np.ascontiguousarray(x.reshape(B * T, C).T.astype(ml_dtypes.bfloat16))
    cosT, sinT = make_tables(T)
    mask4 = make_mask4()
    perm = make_perm()
    in_maps = []
    for c in range(n_cores):
        h0 = 2 * c * HD  # first head's column offset (2 heads per core)
        sl = slice(h0, h0 + 128)
        in_maps.append(
            {
                "xT": xT,
                "wq": np.ascontiguousarray(W_qkv[:, sl].astype(ml_dtypes.bfloat16)),
                "wk": np.ascontiguousarray(
                    W_qkv[:, C:][:, sl].astype(ml_dtypes.bfloat16)
                ),
                "wv": np.ascontiguousarray(
                    W_qkv[:, 2 * C :][:, sl].astype(ml_dtypes.bfloat16)
                ),
                "bq": np.ascontiguousarray(b_qkv[sl].reshape(128, 1)),
                "bk": np.ascontiguousarray(b_qkv[C:][sl].reshape(128, 1)),
                "bv": np.ascontiguousarray(b_qkv[2 * C :][sl].reshape(128, 1)),
                "wp": np.ascontiguousarray(W_proj[sl, :].astype(ml_dtypes.bfloat16)),
                "cosT": np.ascontiguousarray(cosT.astype(ml_dtypes.bfloat16)),
                "sinT": np.ascontiguousarray(sinT.astype(ml_dtypes.bfloat16)),
                "perm": perm.astype(ml_dtypes.bfloat16),
                "mask4": mask4.astype(ml_dtypes.bfloat16),
            }
        )
    return in_maps


_NC_CACHE = {}


def _get_nc(B, T, C):
    key = (B, T, C)
    if key not in _NC_CACHE:
        _NC_CACHE[key] = build_nc(B, T, C)
    return _NC_CACHE[key]


def kernel(x, W_qkv, b_qkv, W_proj, b_proj):
    from concourse.bass_utils import run_bass_kernel_spmd

    x = np.asarray(x, dtype=np.float32)
    W_qkv = np.asarray(W_qkv, dtype=np.float32)
    b_qkv = np.asarray(b_qkv, dtype=np.float32)
    W_proj = np.asarray(W_proj, dtype=np.float32)
    b_proj = np.asarray(b_proj, dtype=np.float32)
    B, T, C = x.shape
    n_cores = 8
    nc = _get_nc(B, T, C)
    in_maps = make_in_maps(x, W_qkv, b_qkv, W_proj, n_cores)
    res = run_bass_kernel_spmd(nc, in_maps, core_ids=list(range(n_cores)))
    out = np.zeros((B * T, C), dtype=np.float32)
    for r in res.results:
        out += r["outp"].astype(np.float32)
    out += b_proj[None, :]
    return out.reshape(B, T, C)
